# revision 22
# baseline (speedup 1.0000x reference)
"""Luong seq2seq (2-layer BiGRU encoder + attention GRU decoder + vocab
projection) as a single 8-core SPMD Bass/Tile kernel for Trainium2.

Sharding: data-parallel over batch (64 examples -> 8 per core). Each core
runs the full recurrence for its 8 examples and projects onto the full
32000-word vocabulary; the host concatenates per-core logits.

v2: keeps the PE warm through the serial gate chain with filler matmuls
that target the next step's PSUM regions, hoists xp/bias/Whd matmuls into
the previous step's window, batches transpose copies, replaces decoder
tanh with 2*sigmoid(2x)-1 (weights pre-scaled on the host) so the ACT
table never reloads, folds the attention mask into the scores matmul,
streams each out_proj chunk once per needed row-tile, and emits bf16
logits (upcast on host).
"""

import os
import sys
import types

for _p in ("/opt/trn_rl_repo", "/opt/pypackages", "/root/.axon_site",
           "/root/.axon_site/_ro/trn_rl_repo", "/root/.axon_site/_ro/pypackages"):
    if os.path.isdir(_p) and _p not in sys.path:
        sys.path.append(_p)

import numpy as np

from concourse import bass, mybir, tile, bacc
from concourse import bass_utils
from concourse.bass_utils import run_bass_kernel_spmd
from concourse.masks import make_identity

# ---------------------------------------------------------------- constants
V, H, T, B, NCORES = 32000, 512, 48, 64, 8
Bc = B // NCORES            # 8 examples per core
H2, H3 = 2 * H, 3 * H
NSEQ = T * Bc               # 384 (t-major row order: r = t*Bc + b)
NSCAN = T * 2 * Bc          # 768 (enc scan rows: r = t*16 + lane*8 + b)
P = 128
NEG = -1.0e9

f32 = mybir.dt.float32
f32r = mybir.dt.float32r
bf16 = mybir.dt.bfloat16
AF = mybir.ActivationFunctionType
OP = mybir.AluOpType

VCHUNKS = [(i * 512, 512) for i in range(62)] + [(62 * 512, 256)]  # 32000

# warm-filler matmul counts (dummies before each prep region open)
ENC_DUM = tuple(int(x) for x in __import__('os').environ.get('ENCDUM','4,4,8').split(','))
DEC_DUM = tuple(int(x) for x in __import__('os').environ.get('DECDUM','3,3,5').split(','))


def _install_profile_hook():
    """Make trace=True work: the image's antenv lacks axon_hooks."""
    if "antenv.axon_hooks" in sys.modules:
        return
    try:
        import trn_agent_boot.trn_boot as tb
        hook = tb._ntff_profile_via_ctypes("/opt/axon/libaxon_pjrt.so")
        m = types.ModuleType("antenv.axon_hooks")
        m.get_axon_ntff_profile_hook = lambda: hook
        m.set_axon_ntff_profile_hook = lambda h: None
        sys.modules["antenv.axon_hooks"] = m
        import antenv
        antenv.axon_hooks = m
        bass_utils.upload_artifacts = lambda d: d
    except Exception:
        pass


# ---------------------------------------------------------------- program
def build_program():
    nc = bacc.Bacc("TRN2", target_bir_lowering=False, debug=False,
                   num_devices=NCORES)

    def din(name, shape, dt=f32r):
        return nc.dram_tensor(name, list(shape), dt, kind="ExternalInput").ap()

    io = {}
    io["xeT_in"] = din("xeT_in", (H, NSCAN))
    io["xdT_in"] = din("xdT_in", (H, NSEQ))
    io["amask"] = din("amask", (Bc, NSEQ))
    for name, shape in [
        ("w0t", (H, H3)), ("u0", (H, H3)), ("b0", (1, H3)), ("bn0", (1, H)),
        ("w1t", (H2, H3)), ("u1", (H, H3)), ("b1", (1, H3)), ("bn1", (1, H)),
        ("fct", (H2, H)), ("fcb", (1, H)), ("was", (H2, H)),
        ("wcc", (H2, H)), ("wch", (H, H)),
        ("wxd", (H, H3)), ("whd", (H, H3)), ("bd", (1, H3)),
        ("ud", (H, H3)), ("bnd", (1, H)),
    ]:
        io[name] = din(name, shape)
    io["owt"] = din("owt", (H, V), bf16)
    io["out"] = nc.dram_tensor("out", [NSEQ, V], bf16, kind="ExternalOutput").ap()

    with tile.TileContext(nc) as tc:
        _emit(nc, tc, io)
    nc.compile()
    return nc


def _emit(nc, tc, io):
    # ------- long-lived pools
    cpool_cm = tc.tile_pool(name="const", bufs=1)
    spool_cm = tc.tile_pool(name="state", bufs=2)
    wpool_cm = tc.tile_pool(name="work", bufs=2)
    wpool4_cm = tc.tile_pool(name="work4", bufs=10)
    xpool_cm = tc.tile_pool(name="xstage", bufs=3)
    dpool_cm = tc.tile_pool(name="dram", bufs=1, space="DRAM")
    pt_cm = tc.tile_pool(name="pt", bufs=2, space="PSUM")
    ps_cm = tc.tile_pool(name="ps", bufs=2, space="PSUM")
    cpool = cpool_cm.__enter__()
    spool = spool_cm.__enter__()
    wpool = wpool_cm.__enter__()
    wpool4 = wpool4_cm.__enter__()
    xpool = xpool_cm.__enter__()
    dpool = dpool_cm.__enter__()
    pt = pt_cm.__enter__()
    ps = ps_cm.__enter__()

    # ---------------- constants
    ident = cpool.tile([P, P], f32)
    make_identity(nc, ident[:])
    identr = cpool.tile([P, P], f32r)
    nc.vector.tensor_copy(identr[:], ident[:])
    ones_f = cpool.tile([1, P], f32)
    nc.vector.memset(ones_f[:], 1.0)
    ones = cpool.tile([1, P], f32r)
    nc.vector.tensor_copy(ones[:], ones_f[:])

    def load_const(name, shape):
        t = cpool.tile(list(shape), f32r, tag=name)
        nc.sync.dma_start(t[:], io[name][:])
        return t

    bn0_sb = load_const("bn0", (1, H))
    bn1_sb = load_const("bn1", (1, H))
    bnd_sb = load_const("bnd", (1, H))
    fcb_sb = load_const("fcb", (1, H))
    amask_sb = cpool.tile([Bc, NSEQ], f32r, tag="amask")
    nc.sync.dma_start(amask_sb[:], io["amask"][:])

    # DRAM scratch
    xp0_d = dpool.tile([NSCAN, H3], f32r)
    xp1_d = dpool.tile([NSCAN, H3], f32r)
    xpx_d = dpool.tile([NSEQ, H3], f32r)
    l0_d = dpool.tile([NSCAN, H2], f32)
    henc_d = dpool.tile([NSEQ, H2], f32)

    # ---------------- helpers
    def kload(pool, name, kdim, n, tag):
        ko = kdim // P
        t = pool.tile([P, ko, n], f32r, tag=tag)
        nc.sync.dma_start(t[:], io[name].rearrange("(ko p) n -> p ko n", p=P))
        return t

    def batched_mm(out_dram, lhsT_tile, kdim, mtiles, rhs_name, nbase,
                   bias_sb, opool):
        """out[m*128.., :nbase] = lhsT.T @ io[rhs_name] + bias -> DRAM f32r.

        Streams the rhs weight in (128, ko, 512) column chunks."""
        ko = kdim // P
        rhs_r = io[rhs_name].rearrange("(ko p) n -> p ko n", p=P)
        for c0 in range(0, nbase, 512):
            cw = min(512, nbase - c0)
            rhs_c = opool.tile([P, ko, 512], f32r, tag="rhsc")
            nc.sync.dma_start(rhs_c[:, :, :cw], rhs_r[:, :, c0:c0 + cw])
            for m in range(mtiles):
                ps_t = ps.tile([P, 512], f32, tag="sc")
                for k in range(ko):
                    nc.tensor.matmul(ps_t[:, :cw],
                                     lhsT_tile[:, k, m * P:(m + 1) * P],
                                     rhs_c[:, k, :cw],
                                     start=(k == 0), stop=False)
                nc.tensor.matmul(ps_t[:, :cw], ones[:1, :P],
                                 bias_sb[:1, c0:c0 + cw],
                                 start=False, stop=True)
                ob = opool.tile([P, 512], f32r, tag="mmob")
                nc.scalar.copy(out=ob[:, :cw], in_=ps_t[:, :cw])
                nc.sync.dma_start(out_dram[m * P:(m + 1) * P, c0:c0 + cw],
                                  ob[:, :cw])

    def transpose_to(dst_ap, src_ap, rows, eng):
        """dst_ap (128, rows) = src_ap (rows, 128) transposed."""
        tp_t = pt.tile([P, P], f32, tag="tp")
        nc.tensor.transpose(tp_t[:, :rows], src_ap, ident[:rows, :rows])
        eng.copy(out=dst_ap, in_=tp_t[:, :rows])

    # =========================================================== gather phase
    gpool_cm = tc.tile_pool(name="gather", bufs=1)
    gwork_cm = tc.tile_pool(name="gwork", bufs=2)
    gpool = gpool_cm.__enter__()
    gwork = gwork_cm.__enter__()

    xeT = kload(gpool, "xeT_in", H, NSCAN, "xeT")
    xdT = kload(gpool, "xdT_in", H, NSEQ, "xdT")

    b0_sb = gpool.tile([1, H3], f32r, tag="b0")
    nc.sync.dma_start(b0_sb[:], io["b0"][:])
    bd_sb2 = gpool.tile([1, H3], f32r, tag="bd")
    nc.sync.dma_start(bd_sb2[:], io["bd"][:])
    batched_mm(xp0_d[:], xeT, H, 6, "w0t", H3, b0_sb, gwork)
    if os.environ.get("KNOXPX", "0") != "1":
        batched_mm(xpx_d[:], xdT, H, 3, "wxd", H3, bd_sb2, gwork)

    gwork_cm.__exit__(None, None, None)
    gpool_cm.__exit__(None, None, None)

    # =========================================================== GRU scan
    def gru_scan(pp, nsteps, m, xp_dram, u_sb, bn_sb, h0_sb, h0T, step_out,
                 sig2, dums):
        """One GRU scan.  m rows per step; 3 PSUM regions (+1 xn for dec).

        Per-iteration emission order (PE queue order):
          [hU MMs (t)] [dummies+prep regions (t+1)] [transposes (t)]
          [step_out.emit(t): attention/whd-prep/proj]
        so the dummies+prep run inside the gate-chain window and the PE
        never sees a >1us idle stretch.
        """
        nreg = 4 if sig2 else 3
        xps = {}

        def load_xp(t):
            xt = xpool.tile([m, H3], f32r, tag="xpt")
            nc.sync.dma_start(xt[:], xp_dram[t * m:(t + 1) * m, :])
            return xt

        def dummy(n, seg):
            for _ in range(n):
                nc.tensor.matmul(seg, identr[:m, :m], u_sb[:m, 0, 0:512],
                                 start=True, stop=True)

        def prep(xp_t, full):
            """Open this step's PSUM regions (xp / bias injections)."""
            p = pp.tile([m, nreg * 512], f32, tag="p")
            nc.tensor.matmul(p[:, 0:512], identr[:m, :m], xp_t[:, 0:512],
                             start=True, stop=full)
            nc.tensor.matmul(p[:, 512:1024], identr[:m, :m],
                             xp_t[:, 512:1024], start=True, stop=full)
            nc.tensor.matmul(p[:, 1024:1536], ones[:1, :m], bn_sb[:1, :],
                             start=True, stop=full)
            if nreg == 4:
                nc.tensor.matmul(p[:, 1536:2048], identr[:m, :m],
                                 xp_t[:, 1024:1536], start=True, stop=False)
            return p

        xps[0] = load_xp(0)
        if nsteps > 1:
            xps[1] = load_xp(1)
        h_sb, hT = h0_sb, h0T

        for t in range(nsteps):
            if t + 2 < nsteps:
                xps[t + 2] = load_xp(t + 2)
            xp_t = xps[t]
            p_cur = prep(xp_t, full=(hT is None and not sig2))
            if sig2:
                step_out.prep_wh(p_cur)
            # ---- 1. recurrent hU matmuls into p_cur
            if hT is not None:
                for c in range(3):
                    seg = p_cur[:, c * 512:(c + 1) * 512]
                    for k in range(4):
                        nc.tensor.matmul(seg, hT[:, k, :],
                                         u_sb[:, k, c * 512:(c + 1) * 512],
                                         start=False, stop=(k == 3))
            # ---- 2. gate chain (ACT: sigr, sigz, act_n; DVE: rn, nin,
            #         omz, hn1, h; GPS: zh)
            rz = wpool.tile([m, H2], f32, tag="rz")
            nc.scalar.activation(rz[:, 0:H], p_cur[:, 0:512], AF.Sigmoid)
            nc.scalar.activation(rz[:, H:H2], p_cur[:, 512:1024], AF.Sigmoid)
            zh = None
            if h_sb is not None:
                zh = wpool4.tile([m, H], f32, tag="g1")
                nc.gpsimd.tensor_mul(zh[:], rz[:, H:H2], h_sb[:])
            rn = wpool4.tile([m, H], f32, tag="g1")
            nc.vector.tensor_mul(rn[:], rz[:, 0:H], p_cur[:, 1024:1536])
            nin = wpool4.tile([m, H], f32, tag="g1")
            if nreg == 4:
                nc.vector.tensor_add(nin[:], rn[:], p_cur[:, 1536:2048])
            else:
                nc.vector.tensor_add(nin[:], rn[:], xp_t[:, 1024:1536])
            omz = wpool4.tile([m, H], f32, tag="g1")
            nc.vector.tensor_scalar(omz[:], rz[:, H:H2], -1.0, 1.0,
                                    OP.mult, OP.add)
            h_new = spool.tile([m, H], f32, tag="h")
            if sig2:
                # n = tanh(nin) = 2*sig(2*nin)-1 -- keeps the ACT table on
                # {Sigmoid, Exp} so it never reloads during decode.
                s_n = wpool4.tile([m, H], f32, tag="g1")
                nc.scalar.activation(s_n[:], nin[:], AF.Sigmoid, scale=2.0)
                zhm = wpool4.tile([m, H], f32, tag="g1")
                nc.vector.tensor_sub(zhm[:], zh[:], omz[:])
                hn1 = wpool4.tile([m, H], f32, tag="g1")
                # (s*2) * omz
                nc.vector.scalar_tensor_tensor(hn1[:], s_n[:], 2.0, omz[:],
                                               OP.mult, OP.mult)
                nc.vector.tensor_add(h_new[:], hn1[:], zhm[:])
            else:
                n_t = wpool4.tile([m, H], f32, tag="g1")
                nc.scalar.activation(n_t[:], nin[:], AF.Tanh)
                if h_sb is not None:
                    hn1 = wpool4.tile([m, H], f32, tag="g1")
                    nc.vector.tensor_mul(hn1[:], omz[:], n_t[:])
                    nc.vector.tensor_add(h_new[:], hn1[:], zh[:])
                else:
                    nc.vector.tensor_mul(h_new[:], omz[:], n_t[:])
            # ---- 3. warm fillers: dummy matmuls into the regions the gate
            #         chain has already consumed (WAR-ordered to run inside
            #         the chain window, keeping the PE HAM clock at 2.4 GHz)
            dummy(dums[0], p_cur[:, 0:512])
            dummy(dums[1], p_cur[:, 512:1024])
            dummy(dums[2], p_cur[:, 1024:1536])
            # ---- 4. transposes: h_new (m, 512) -> hT_new (128, 4, m)
            tp_t = pt.tile([P, P], f32, tag="tp")
            for k in range(4):
                nc.tensor.transpose(tp_t[:, k * m:(k + 1) * m],
                                    h_new[:, k * P:(k + 1) * P],
                                    ident[:m, :m])
            hT_new = spool.tile([P, 4, m], f32r, tag="hT")
            if os.environ.get("KNOREARR", "0") == "1":
                for k in range(4):
                    nc.scalar.copy(out=hT_new[:, k, :],
                                   in_=tp_t[:, k * m:(k + 1) * m])
            else:
                nc.scalar.copy(out=hT_new[:],
                               in_=tp_t[:, 0:4 * m].rearrange(
                                   "p (k m) -> p k m", k=4))
            step_out.emit(t, h_new, hT_new)
            h_sb, hT = h_new, hT_new
        return h_sb, hT

    class EncOut:
        def __init__(self, kind):
            self.kind = kind

        def emit(self, s, h_new, hT_new):
            if self.kind == "l0":
                d = l0_d
                nc.sync.dma_start(d[s * 16:s * 16 + 8, 0:H], h_new[0:8, :])
                nc.sync.dma_start(
                    d[(T - 1 - s) * 16 + 8:(T - 1 - s) * 16 + 16, 0:H],
                    h_new[0:8, :])
                nc.sync.dma_start(
                    d[(T - 1 - s) * 16:(T - 1 - s) * 16 + 8, H:H2],
                    h_new[8:16, :])
                nc.sync.dma_start(d[s * 16 + 8:s * 16 + 16, H:H2],
                                  h_new[8:16, :])
            else:
                d = henc_d
                nc.sync.dma_start(d[s * Bc:(s + 1) * Bc, 0:H], h_new[0:8, :])
                nc.sync.dma_start(d[(T - 1 - s) * Bc:(T - s) * Bc, H:H2],
                                  h_new[8:16, :])

    # ---- encoder
    e0pool_cm = tc.tile_pool(name="encp", bufs=1)
    e0work_cm = tc.tile_pool(name="encw", bufs=2)
    ppE_cm = tc.tile_pool(name="ppE", bufs=1, space="PSUM")
    e0pool = e0pool_cm.__enter__()
    e0work = e0work_cm.__enter__()
    ppE = ppE_cm.__enter__()

    TRUNC = int(os.environ.get("KTRUNC", "9"))

    def trunc_out():
        zt = cpool.tile([P, 512], bf16, tag="zt")
        nc.vector.memset(zt[:], 0.0)
        nc.sync.dma_start(io["out"][0:P, 0:512], zt[:])

    u0_sb = kload(e0pool, "u0", H, H3, "u0sb")
    gru_scan(ppE, T, 16, xp0_d[:], u0_sb, bn0_sb, None, None, EncOut("l0"),
             sig2=False, dums=ENC_DUM)
    if TRUNC <= 1:
        trunc_out()
        ppE_cm.__exit__(None, None, None)
        e0work_cm.__exit__(None, None, None)
        e0pool_cm.__exit__(None, None, None)
        for cm in (ps_cm, pt_cm, dpool_cm, xpool_cm, wpool4_cm, wpool_cm,
                   spool_cm, cpool_cm):
            cm.__exit__(None, None, None)
        return

    l0T = e0pool.tile([P, 8, NSCAN], f32r, tag="l0T")
    for mm in range(6):
        lrow = e0work.tile([P, H2], f32, tag="lrow")
        nc.sync.dma_start(lrow[:], l0_d[mm * P:(mm + 1) * P, :])
        for k in range(8):
            transpose_to(l0T[:, k, mm * P:(mm + 1) * P],
                         lrow[:, k * P:(k + 1) * P], P, nc.scalar)
    b1_sb = e0pool.tile([1, H3], f32r, tag="b1")
    nc.sync.dma_start(b1_sb[:], io["b1"][:])
    batched_mm(xp1_d[:], l0T, H2, 6, "w1t", H3, b1_sb, e0work)

    u1_sb = kload(e0pool, "u1", H, H3, "u0sb")   # reuse u0 slot
    gru_scan(ppE, T, 16, xp1_d[:], u1_sb, bn1_sb, None, None, EncOut("henc"),
             sig2=False, dums=ENC_DUM)

    ppE_cm.__exit__(None, None, None)
    e0work_cm.__exit__(None, None, None)
    e0pool_cm.__exit__(None, None, None)
    if TRUNC <= 2:
        trunc_out()
        for cm in (ps_cm, pt_cm, dpool_cm, xpool_cm, wpool4_cm, wpool_cm,
                   spool_cm, cpool_cm):
            cm.__exit__(None, None, None)
        return

    # =========================================================== attention pre
    mpool_cm = tc.tile_pool(name="mid", bufs=1)
    mwork_cm = tc.tile_pool(name="midw", bufs=2)
    mpool = mpool_cm.__enter__()
    mwork = mwork_cm.__enter__()

    hencT = mpool.tile([P, 8, NSEQ], f32r, tag="hencT")
    for mm in range(3):
        hrow = mwork.tile([P, H2], f32, tag="hrow")
        nc.sync.dma_start(hrow[:], henc_d[mm * P:(mm + 1) * P, :])
        for k in range(8):
            transpose_to(hencT[:, k, mm * P:(mm + 1) * P],
                         hrow[:, k * P:(k + 1) * P], P, nc.scalar)

    was_sb = kload(mwork, "was", H2, H, "wpre")
    gT = mpool.tile([P, 4, NSEQ], f32r, tag="gT")
    for mm in range(4):
        ps_t = ps.tile([P, 512], f32, tag="sc")
        for k in range(8):
            nc.tensor.matmul(ps_t[:, :NSEQ], was_sb[:, k, mm * P:(mm + 1) * P],
                             hencT[:, k, :], start=(k == 0), stop=(k == 7))
        nc.scalar.copy(out=gT[:, mm, :], in_=ps_t[:, :NSEQ])

    wcc_sb = kload(mwork, "wcc", H2, H, "wpre")
    pf = mpool.tile([P, 3, H], f32r, tag="pf")
    for mm in range(3):
        ps_t = ps.tile([P, 512], f32, tag="sc")
        for k in range(8):
            nc.tensor.matmul(ps_t[:, :H], hencT[:, k, mm * P:(mm + 1) * P],
                             wcc_sb[:, k, :], start=(k == 0), stop=(k == 7))
        nc.scalar.copy(out=pf[:, mm, :], in_=ps_t[:, :H])

    fct_sb = kload(mwork, "fct", H2, H, "wpre")
    h0p = ps.tile([P, 512], f32, tag="sc")
    for k in range(8):
        c0 = (T - 1) * Bc if k < 4 else 0
        nc.tensor.matmul(h0p[:Bc, :H], hencT[:, k, c0:c0 + Bc],
                         fct_sb[:, k, :], start=(k == 0), stop=False)
    nc.tensor.matmul(h0p[:Bc, :H], ones[:1, :Bc], fcb_sb[:1, :],
                     start=False, stop=True)
    h0_sb = spool.tile([Bc, H], f32, tag="h")
    nc.scalar.activation(h0_sb[:], h0p[:Bc, :H], AF.Tanh)
    h0T = spool.tile([P, 4, Bc], f32r, tag="hT")
    tp0 = pt.tile([P, P], f32, tag="tp")
    for k in range(4):
        nc.tensor.transpose(tp0[:, k * Bc:(k + 1) * Bc],
                            h0_sb[:, k * P:(k + 1) * P], ident[:Bc, :Bc])
    nc.scalar.copy(out=h0T[:],
                   in_=tp0[:, 0:4 * Bc].rearrange("p (k m) -> p k m", k=4))

    # =========================================================== decoder
    ud_sb = kload(mpool, "ud", H, H3, "udsb")
    whd_sb = kload(mpool, "whd", H, H3, "whdsb")   # pre-scaled 2*Whd
    wch_sb = kload(mpool, "wch", H, H, "wchsb")
    htall = mpool.tile([P, 4, NSEQ], bf16, tag="htall")

    # initial attention state: ht0 = 0  =>  s_ht0 = 0.5 everywhere
    sht0_f = mpool.tile([P, 4, Bc], f32, tag="sht0f")
    nc.vector.memset(sht0_f[:], 0.5)
    sht0_T = mpool.tile([P, 4, Bc], f32r, tag="sht0")
    nc.vector.tensor_copy(sht0_T[:], sht0_f[:])

    prpool_cm = tc.tile_pool(name="proj", bufs=3)
    prpool = prpool_cm.__enter__()
    owt_r = io["owt"].rearrange("(ko p) v -> p ko v", p=P)

    def emit_proj(mtiles, c0, cw):
        ow = prpool.tile([P, 4, 512], bf16, tag="ow")
        nc.sync.dma_start(ow[:, :, :cw], owt_r[:, :, c0:c0 + cw])
        for mm in mtiles:
            ps_t = ps.tile([P, 512], f32, tag="sc")
            for k in range(4):
                nc.tensor.matmul(ps_t[:, :cw], htall[:, k, mm * P:(mm + 1) * P],
                                 ow[:, k, :cw], start=(k == 0), stop=(k == 3))
            ob = prpool.tile([P, 512], bf16, tag="ob")
            nc.vector.tensor_copy(out=ob[:, :cw], in_=ps_t[:, :cw])
            nc.sync.dma_start(io["out"][mm * P:(mm + 1) * P, c0:c0 + cw],
                              ob[:, :cw])

    m0_chunks = list(VCHUNKS)

    class DecOut:
        def __init__(self):
            self.ht_T = sht0_T

        def prep_wh(self, p):
            """2*Whd matmuls for the next step's regions (r, z, xn)."""
            for c, (w0, stop) in enumerate([(0, False), (512, False),
                                            (1024, True)]):
                seg = p[:, c * 512:(c + 1) * 512] if c < 2 \
                    else p[:, 1536:2048]
                for k in range(4):
                    nc.tensor.matmul(seg, self.ht_T[:, k, :],
                                     whd_sb[:, k, w0:w0 + 512],
                                     start=False, stop=(stop and k == 3))

        def emit(self, t, h_new, hT_new):
            # ---- scores = h@Wa . Henc + mask  (mask folded into PSUM)
            sc = ps.tile([P, 512], f32, tag="sc")
            nc.tensor.matmul(sc[:Bc, :NSEQ], identr[:Bc, :Bc], amask_sb[:],
                             start=True, stop=False)
            for k in range(4):
                nc.tensor.matmul(sc[:Bc, :NSEQ], hT_new[:, k, :],
                                 gT[:, k, :], start=False, stop=(k == 3))
            alpha = wpool.tile([Bc, NSEQ], f32, tag="alpha")
            sexp = wpool.tile([Bc, 1], f32, tag="sexp")
            nc.scalar.activation(alpha[:], sc[:Bc, :NSEQ], AF.Exp,
                                 accum_out=sexp[:])
            rs = wpool.tile([Bc, 1], f32, tag="rs")
            nc.vector.reciprocal(rs[:], sexp[:])
            nc.vector.tensor_scalar_mul(alpha[:], alpha[:], rs[:])
            tpa = pt.tile([P, P], f32, tag="tp")
            for j in range(3):
                nc.tensor.transpose(tpa[:, j * Bc:(j + 1) * Bc],
                                    alpha[:, j * P:(j + 1) * P],
                                    ident[:Bc, :Bc])
            aT = wpool.tile([P, 3, Bc], f32r, tag="aT")
            nc.scalar.copy(out=aT[:],
                           in_=tpa[:, 0:3 * Bc].rearrange(
                               "p (k m) -> p k m", k=3))
            # ---- ht = tanh(h@Wch + ctx@Wcc) via 2*sig(2x)-1
            htp = ps.tile([P, 512], f32, tag="sc")
            for j in range(3):
                nc.tensor.matmul(htp[:Bc, :H], aT[:, j, :], pf[:, j, :],
                                 start=(j == 0), stop=False)
            for k in range(4):
                nc.tensor.matmul(htp[:Bc, :H], hT_new[:, k, :],
                                 wch_sb[:, k, :], start=False, stop=(k == 3))
            s_ht = wpool4.tile([Bc, H], f32, tag="g1")
            nc.scalar.activation(s_ht[:], htp[:Bc, :H], AF.Sigmoid, scale=2.0)
            tph = pt.tile([P, P], f32, tag="tp")
            for k in range(4):
                nc.tensor.transpose(tph[:, k * Bc:(k + 1) * Bc],
                                    s_ht[:, k * P:(k + 1) * P],
                                    ident[:Bc, :Bc])
            ht_T = spool.tile([P, 4, Bc], f32r, tag="htT")
            nc.scalar.copy(out=ht_T[:],
                           in_=tph[:, 0:4 * Bc].rearrange(
                               "p (k m) -> p k m", k=4))
            # real ht = 2*s - 1 into htall (bf16) for the vocab projection
            nc.vector.tensor_scalar(
                htall[:, :, t * Bc:(t + 1) * Bc],
                tph[:, 0:4 * Bc].rearrange("p (k m) -> p k m", k=4),
                2.0, -1.0, OP.mult, OP.add)
            self.ht_T = ht_T
            # ---- interleaved vocab projection for row-tile 0
            if t >= 16:
                for _ in range(2):
                    if m0_chunks:
                        c0, cw = m0_chunks.pop(0)
                        emit_proj([0], c0, cw)

    if TRUNC <= 3:
        trunc_out()
        prpool_cm.__exit__(None, None, None)
        mwork_cm.__exit__(None, None, None)
        mpool_cm.__exit__(None, None, None)
        for cm in (ps_cm, pt_cm, dpool_cm, xpool_cm, wpool4_cm, wpool_cm,
                   spool_cm, cpool_cm):
            cm.__exit__(None, None, None)
        return

    ppD_cm = tc.tile_pool(name="ppD", bufs=1, space="PSUM")
    ppD = ppD_cm.__enter__()
    dec_out = DecOut()
    gru_scan(ppD, T, Bc, xpx_d[:], ud_sb, bnd_sb, h0_sb, h0T, dec_out,
             sig2=True, dums=DEC_DUM)
    ppD_cm.__exit__(None, None, None)

    # ---- projection tail: leftover m0 chunks, then m1+m2 chunk-major
    while m0_chunks:
        c0, cw = m0_chunks.pop(0)
        emit_proj([0], c0, cw)
    for c0, cw in VCHUNKS:
        emit_proj([1, 2], c0, cw)

    prpool_cm.__exit__(None, None, None)
    mwork_cm.__exit__(None, None, None)
    mpool_cm.__exit__(None, None, None)

    for cm in (ps_cm, pt_cm, dpool_cm, xpool_cm, wpool4_cm, wpool_cm,
               spool_cm, cpool_cm):
        cm.__exit__(None, None, None)


# ---------------------------------------------------------------- host side
_PROGRAM = None


def _get_program():
    global _PROGRAM
    if _PROGRAM is None:
        _install_profile_hook()
        _PROGRAM = build_program()
    return _PROGRAM


def _prep_shared(inputs):
    f = np.float32
    g = {}
    g["w0t"] = np.ascontiguousarray(np.asarray(inputs["enc0_Wih"], f).T)
    g["u0"] = np.concatenate([np.asarray(inputs["enc0_Ur"], f).T,
                              np.asarray(inputs["enc0_Uz"], f).T,
                              np.asarray(inputs["enc0_Un"], f).T], axis=1)
    g["b0"] = np.asarray(inputs["enc0_bih"], f)[None, :]
    g["bn0"] = np.asarray(inputs["enc0_bn"], f)[None, :]
    g["w1t"] = np.ascontiguousarray(np.asarray(inputs["enc1_Wih"], f).T)
    g["u1"] = np.concatenate([np.asarray(inputs["enc1_Ur"], f).T,
                              np.asarray(inputs["enc1_Uz"], f).T,
                              np.asarray(inputs["enc1_Un"], f).T], axis=1)
    g["b1"] = np.asarray(inputs["enc1_bih"], f)[None, :]
    g["bn1"] = np.asarray(inputs["enc1_bn"], f)[None, :]
    g["fct"] = np.ascontiguousarray(np.asarray(inputs["fc_init_w"], f).T)
    g["fcb"] = np.asarray(inputs["fc_init_b"], f)[None, :]
    scale = np.float32(1.0) / np.sqrt(np.float32(H2))
    g["was"] = np.ascontiguousarray(np.asarray(inputs["Wa"], f) * scale)
    acw = np.asarray(inputs["attn_combine_w"], f)
    g["wch"] = np.ascontiguousarray(acw[:, :H].T)
    g["wcc"] = np.ascontiguousarray(acw[:, H:].T)
    dwih = np.asarray(inputs["dec_Wih"], f)
    g["wxd"] = np.ascontiguousarray(dwih[:, :H].T)
    # decoder attention-state trick: ht = 2*s - 1 with s = sig(2x), so use
    # 2*Whd as the weight and fold the "-1" row-sum into the bias.
    whd = dwih[:, H:]                                    # (3H, H)
    g["whd"] = np.ascontiguousarray((2.0 * whd).T)
    g["bd"] = (np.asarray(inputs["dec_bih"], f)
               - whd.sum(axis=1))[None, :]
    g["ud"] = np.concatenate([np.asarray(inputs["dec_Ur"], f).T,
                              np.asarray(inputs["dec_Uz"], f).T,
                              np.asarray(inputs["dec_Un"], f).T], axis=1)
    g["bnd"] = np.asarray(inputs["dec_bn"], f)[None, :]
    import ml_dtypes
    g["owt"] = np.ascontiguousarray(
        np.asarray(inputs["out_w"], f).T.astype(ml_dtypes.bfloat16))
    for k in g:
        g[k] = np.ascontiguousarray(g[k])
    return g


def _prep_core(inputs, c):
    src = np.asarray(inputs["src"])
    tgt = np.asarray(inputs["tgt"])
    emb = np.asarray(inputs["emb"], np.float32)
    si = src[:, c * Bc:(c + 1) * Bc].astype(np.int64)      # (48, 8)
    ti = tgt[:, c * Bc:(c + 1) * Bc].astype(np.int64)
    idx_enc = np.empty((T, 2, Bc), np.int64)
    idx_enc[:, 0, :] = si
    idx_enc[:, 1, :] = si[::-1]
    xeT_in = np.ascontiguousarray(emb[idx_enc.reshape(NSCAN)].T)
    xdT_in = np.ascontiguousarray(emb[ti.reshape(NSEQ)].T)
    m = np.full((Bc, T, Bc), NEG, np.float32)
    for b in range(Bc):
        m[b, :, b] = np.where(si[:, b] != 0, np.float32(0.0), np.float32(NEG))
    return {"xeT_in": xeT_in,
            "xdT_in": xdT_in,
            "amask": m.reshape(Bc, NSEQ)}


def kernel(**inputs):
    nc = _get_program()
    shared = _prep_shared(inputs)
    in_maps = []
    for c in range(NCORES):
        im = dict(shared)
        im.update(_prep_core(inputs, c))
        in_maps.append(im)
    res = run_bass_kernel_spmd(nc, in_maps, core_ids=list(range(NCORES)))
    logits = np.empty((T, B, V), np.float32)
    for c in range(NCORES):
        logits[:, c * Bc:(c + 1) * Bc, :] = \
            res.results[c]["out"].astype(np.float32).reshape(T, Bc, V)
    return logits
